# revision 11
# baseline (speedup 1.0000x reference)
"""Tensor-parallel multi-head attention for Trainium2 (8 NeuronCores).

Reference computation (fp32):
    qkv = hidden @ w_qkv.T + b_qkv            # [B,S,3H]
    q,k,v = split/heads                       # [B,NH,S,HD]
    out_h = softmax(q k^T / sqrt(HD)) v       # [B,NH,S,HD]
    out = concat_heads(out_h) @ w_out.T + b_out

Sharding (Megatron-style tensor parallel over NH=16 heads, 2 heads/core):
  - hidden (transposed, [H, B*S]) replicated to all 8 cores
  - each core: QKV projection for its 2 heads -> attention for its 2 heads
    -> unnormalized context^T [256, 4096] (softmax denominators folded in
    before the gather)
  - AllGather context^T over cores -> [2048, 4096]
  - each core computes a disjoint 256-column slice of the output projection
  - host concatenates column slices: zero host FLOPs

Layout choices keep every operand in the orientation its matmul needs, so
no on-device transposes are required anywhere:
  - QKV products are produced as qkv^T ([out_dim, token]) for Q/K by making
    the weight the stationary operand; V is produced in natural [token, d]
    layout by making the activation the stationary operand.
  - scores are produced transposed (scores^T[k, q] = K Q^T), so the
    attn@V contraction over k has k on partitions for both operands.
  - softmax over k (= partitions of scores^T): exp via ACT, per-column sums
    via a DVE accumulation over k-tiles + one ones-vector matmul
    (partition reduce), reciprocal on DVE, partition-broadcast via a K=1
    ones matmul, and one multiply on the (16x smaller) context.
"""

import sys

sys.path.insert(0, "/opt/trn_rl_repo")

import numpy as np

import concourse.bass as bass
import concourse.tile as tile
from concourse import mybir
from concourse.bass_utils import run_bass_kernel_spmd
from concourse.tile import ScopedClock

FP32 = mybir.dt.float32

B = 2
S = 2048
H = 2048
NH = 16
HD = 128
N_CORES = 8
HPC = NH // N_CORES  # heads per core = 2
T = B * S  # 4096
O_QK = 2 * HPC * HD  # 512 rows of qk^T per core (Q then K)
O_V = HPC * HD  # 256
O_OUT = H // N_CORES  # 256 output columns per core
SCALE = 1.0 / float(np.sqrt(HD))
P = 128

# matmul dtype for each stage; float32 is exact, float32r trades precision
# for 4x PE throughput (tf32-like). Chosen empirically; see test.py.
MM_DT = FP32


MAX_WAITS = 1  # the pinned walrus codegen rejects >1 sync wait per inst


def _wait_limit(inst):
    return MAX_WAITS


class _TileContext(tile.TileContext):
    """Tile patched for the pinned walrus codegen's sync-wait limit.

    Any instruction carrying more than MAX_WAITS semaphore waits is split:
    preceding same-engine nops carry the excess (engines execute their
    stream in order, so the waits still all precede the instruction).
    """

    def _lower_ordered_insts(self, ordered):
        nc = self.nc
        for bb_name, insts in list(ordered.items()):
            new_insts = []
            for inst in insts:
                si = inst.sync_info
                limit = _wait_limit(inst)
                if (
                    si is not None
                    and len(si.on_wait) > limit
                    and inst.engine is not None
                ):
                    waits = list(si.on_wait)
                    while len(waits) > limit:
                        chunk, waits = waits[:limit], waits[limit:]
                        new_insts.append(
                            mybir.InstNoOp(
                                name=nc.get_next_instruction_name(),
                                sync_info=mybir.SyncInfo(
                                    on_wait=chunk, on_update=[]
                                ),
                                bass_nofuse=True,
                                engine=inst.engine,
                            )
                        )
                    inst.sync_info = mybir.SyncInfo(
                        on_wait=waits, on_update=list(si.on_update)
                    )
                new_insts.append(inst)
            ordered[bb_name] = new_insts
        return super()._lower_ordered_insts(ordered)

    def _drain_and_barrier(self, tick_clock, wait_clock):
        nc = self.nc
        probe = nc.sync.nop(nofuse=True, hint="drain_wait_probe")
        wait_clock.add_sem_waits(
            probe.ins, ScopedClock({None: tick_clock.global_clock})
        )
        si = probe.ins.sync_info
        waits = list(si.on_wait) if si is not None else []
        probe.ins.sync_info = mybir.SyncInfo(
            on_wait=[], on_update=list(si.on_update) if si else []
        )
        for w in waits:
            n = nc.sync.nop(nofuse=True, hint="drain_wait_split")
            n.ins.sync_info = mybir.SyncInfo(on_wait=[w], on_update=[])
        nc.sync.drain()
        nc.all_engine_barrier()
        assert self.sems is not None
        popped = nc._tile_sem_poison_stack.pop()
        assert popped is self._sem_poison
        nc.clear_and_free_semaphores(list(self.sems.allocated().values()))
        nc.all_engine_barrier()


def _build_program(seq=S, mm_dt=MM_DT):
    """Build the SPMD Bass program (identical on all 8 cores)."""
    t_all = B * seq
    n_ht = H // P  # 16 k-tiles over the hidden dim
    ts_w = 256  # token-slice width for the QKV stage
    n_ts = t_all // ts_w
    qs_w = 512 if seq % 512 == 0 else 256  # q-slice width in attention
    n_qs = seq // qs_w
    n_kt = seq // P  # k tiles per batch in attention
    n_tt = t_all // P
    n_dt = H // P  # d tiles of the gathered context
    tg_w = 512  # token group width in the output projection
    n_tg = t_all // tg_w

    nc = bass.Bass(
        "TRN2", target_bir_lowering=False, debug=False, num_devices=N_CORES
    )

    xt = nc.dram_tensor("xt", [H, t_all], FP32, kind="ExternalInput")
    w1t_qk = nc.dram_tensor("w1t_qk", [H, O_QK], FP32, kind="ExternalInput")
    w1t_v = nc.dram_tensor("w1t_v", [H, O_V], FP32, kind="ExternalInput")
    b_qk = nc.dram_tensor("b_qk", [P, O_QK // P], FP32, kind="ExternalInput")
    b_v = nc.dram_tensor("b_v", [P, O_V], FP32, kind="ExternalInput")
    wout_t = nc.dram_tensor("wout_t", [H, O_OUT], FP32, kind="ExternalInput")
    b_out = nc.dram_tensor("b_out", [P, O_OUT], FP32, kind="ExternalInput")
    out = nc.dram_tensor("out", [t_all, O_OUT], FP32, kind="ExternalOutput")

    cc_in = nc.dram_tensor("cc_in", [O_V, t_all], FP32)
    cc_out = nc.dram_tensor("cc_out", [H, t_all], FP32, addr_space="Shared")

    xt_r = xt.ap().rearrange("(kt p) t -> p kt t", p=P)
    w1t_qk_r = w1t_qk.ap().rearrange("(kt p) o -> p kt o", p=P)
    w1t_v_r = w1t_v.ap().rearrange("(kt p) o -> p kt o", p=P)
    wout_r = wout_t.ap().rearrange("(dt p) o -> p dt o", p=P)
    cc_in_r = cc_in.ap().rearrange("(h p) t -> p h t", p=P)
    cc_out_r = cc_out.ap().rearrange("(dt p) t -> p dt t", p=P)
    out_r = out.ap().rearrange("(tt p) o -> p tt o", p=P)

    with _TileContext(nc) as tc:
        with tc.tile_pool(name="const", bufs=1) as const:
            b_qk_sb = const.tile([P, O_QK // P], FP32)
            nc.sync.dma_start(b_qk_sb[:], b_qk.ap())
            b_v_sb = const.tile([P, O_V], FP32)
            nc.sync.dma_start(b_v_sb[:], b_v.ap())
            b_out_sb = const.tile([P, O_OUT], FP32)
            nc.sync.dma_start(b_out_sb[:], b_out.ap())
            ones_col = const.tile([P, 1], mm_dt)
            nc.vector.memset(ones_col[:], 1.0)
            ones_row = const.tile([1, P], FP32)
            nc.vector.memset(ones_row[:], 1.0)

            # ---------------- Stages 1+2 (qk^T and V resident) ----------
            acts_scope = tc.tile_pool(name="acts", bufs=1)
            acts = acts_scope.__enter__()
            qk_sb = acts.tile([P, O_QK // P, t_all], mm_dt)  # qk^T
            v_sb = acts.tile([P, n_tt, O_V], mm_dt)  # V natural

            # ---------------- Stage 1: QKV projection ----------------
            with (
                tc.tile_pool(name="wq", bufs=1) as wq,
                tc.tile_pool(name="xts", bufs=3) as xts,
                tc.tile_pool(name="ps1", bufs=2, space="PSUM") as ps1,
                tc.tile_pool(name="ps1v", bufs=2, space="PSUM") as ps1v,
            ):
                w_qk_sb = wq.tile([P, n_ht, O_QK], mm_dt)
                nc.sync.dma_start(w_qk_sb[:], w1t_qk_r)
                w_v_sb = wq.tile([P, n_ht, O_V], mm_dt)
                nc.sync.dma_start(w_v_sb[:], w1t_v_r)

                for ts_i in range(n_ts):
                    xt_t = xts.tile([P, n_ht, ts_w], mm_dt)
                    nc.sync.dma_start(
                        xt_t[:], xt_r[:, :, ts_i * ts_w : (ts_i + 1) * ts_w]
                    )
                    for ot in range(O_QK // P):
                        ps = ps1.tile([P, ts_w], FP32)
                        for kt in range(n_ht):
                            nc.tensor.matmul(
                                ps[:],
                                w_qk_sb[:, kt, ot * P : (ot + 1) * P],
                                xt_t[:, kt, :],
                                start=(kt == 0),
                                stop=(kt == n_ht - 1),
                            )
                        nc.scalar.activation(
                            qk_sb[:, ot, ts_i * ts_w : (ts_i + 1) * ts_w],
                            ps[:],
                            mybir.ActivationFunctionType.Identity,
                            bias=b_qk_sb[:, ot : ot + 1],
                        )
                    for tt in range(ts_w // P):
                        psv = ps1v.tile([P, O_V], FP32)
                        for kt in range(n_ht):
                            nc.tensor.matmul(
                                psv[:],
                                xt_t[:, kt, tt * P : (tt + 1) * P],
                                w_v_sb[:, kt, :],
                                start=(kt == 0),
                                stop=(kt == n_ht - 1),
                            )
                        nc.vector.tensor_add(
                            v_sb[:, ts_i * (ts_w // P) + tt, :], psv[:], b_v_sb[:]
                        )

            # ---------------- Stage 2: attention (2 heads) ----------------
            with (
                tc.tile_pool(name="ctxp", bufs=1) as ctxp,
                tc.tile_pool(name="exps", bufs=4) as exps,
                tc.tile_pool(name="accs", bufs=2) as accs,
                tc.tile_pool(name="sums", bufs=2) as sums,
                tc.tile_pool(name="ps_s", bufs=2, space="PSUM") as ps_s_pool,
                tc.tile_pool(name="ps_c", bufs=2, space="PSUM") as ps_c_pool,
                tc.tile_pool(name="ps_r", bufs=2, space="PSUM") as ps_r_pool,
            ):
                ctx_sb = ctxp.tile([P, HPC, t_all], FP32)  # context^T
                for b in range(B):
                    for h in range(HPC):
                        for qs in range(n_qs):
                            q_lo = b * seq + qs * qs_w
                            ps_ctx = ps_c_pool.tile([P, qs_w], FP32)
                            acc = accs.tile([P, qs_w], FP32)
                            for kt in range(n_kt):
                                k_lo = b * seq + kt * P
                                ps_sc = ps_s_pool.tile([P, qs_w], FP32)
                                nc.tensor.matmul(
                                    ps_sc[:],
                                    qk_sb[:, HPC + h, k_lo : k_lo + P],
                                    qk_sb[:, h, q_lo : q_lo + qs_w],
                                    start=True,
                                    stop=True,
                                )
                                exp_t = exps.tile([P, qs_w], mm_dt)
                                nc.scalar.activation(
                                    exp_t[:],
                                    ps_sc[:],
                                    mybir.ActivationFunctionType.Exp,
                                    scale=SCALE,
                                )
                                if kt == 0:
                                    nc.vector.tensor_copy(acc[:], exp_t[:])
                                else:
                                    nc.vector.tensor_add(acc[:], acc[:], exp_t[:])
                                nc.tensor.matmul(
                                    ps_ctx[:],
                                    v_sb[
                                        :,
                                        (b * seq) // P + kt,
                                        h * HD : (h + 1) * HD,
                                    ],
                                    exp_t[:],
                                    start=(kt == 0),
                                    stop=(kt == n_kt - 1),
                                )
                            # softmax denominators: partition-reduce acc,
                            # reciprocal, broadcast back over partitions
                            ps_sum = ps_r_pool.tile([1, qs_w], FP32)
                            nc.tensor.matmul(
                                ps_sum[:], ones_col[:], acc[:], start=True, stop=True
                            )
                            inv = sums.tile([1, qs_w], FP32)
                            nc.vector.reciprocal(inv[:], ps_sum[:])
                            ps_b = ps_r_pool.tile([P, qs_w], FP32)
                            nc.tensor.matmul(
                                ps_b[:], ones_row[:], inv[:], start=True, stop=True
                            )
                            invb = sums.tile([P, qs_w], FP32)
                            nc.vector.tensor_copy(invb[:], ps_b[:])
                            nc.vector.tensor_mul(
                                ctx_sb[:, h, q_lo : q_lo + qs_w],
                                ps_ctx[:],
                                invb[:],
                            )

                # stash context^T to DRAM for the gather (inside the scope
                # so the pool free waits on this read)
                nc.sync.dma_start(cc_in_r, ctx_sb[:])

            # release qk^T / V buffers before stage 3 allocates
            acts_scope.__exit__(None, None, None)

            # gather context across cores
            nc.gpsimd.collective_compute(
                "AllGather",
                mybir.AluOpType.bypass,
                replica_groups=[list(range(N_CORES))],
                ins=[cc_in.ap()],
                outs=[cc_out.ap()],
            )

            # ---------------- Stage 3: output projection ----------------
            with (
                tc.tile_pool(name="wo", bufs=1) as wo,
                tc.tile_pool(name="ctxs", bufs=3) as ctxs,
                tc.tile_pool(name="outs", bufs=3) as outs,
                tc.tile_pool(name="ps3", bufs=2, space="PSUM") as ps3,
            ):
                wout_sb = wo.tile([P, n_dt, O_OUT], mm_dt)
                nc.sync.dma_start(wout_sb[:], wout_r)
                for tg in range(n_tg):
                    ctx_t = ctxs.tile([P, n_dt, tg_w], mm_dt)
                    nc.sync.dma_start(
                        ctx_t[:], cc_out_r[:, :, tg * tg_w : (tg + 1) * tg_w]
                    )
                    for tt in range(tg_w // P):
                        ps_o = ps3.tile([P, O_OUT], FP32)
                        for dt in range(n_dt):
                            nc.tensor.matmul(
                                ps_o[:],
                                ctx_t[:, dt, tt * P : (tt + 1) * P],
                                wout_sb[:, dt, :],
                                start=(dt == 0),
                                stop=(dt == n_dt - 1),
                            )
                        out_t = outs.tile([P, O_OUT], FP32)
                        nc.vector.tensor_add(out_t[:], ps_o[:], b_out_sb[:])
                        nc.sync.dma_start(
                            out_r[:, tg * (tg_w // P) + tt, :], out_t[:]
                        )

    return nc


def _make_in_maps(hidden_states, w_qkv, b_qkv, w_out, b_out):
    b, s, _ = hidden_states.shape
    t_all = b * s
    x = np.ascontiguousarray(
        hidden_states.reshape(t_all, H).T, dtype=np.float32
    )  # [H, T]
    in_maps = []
    for c in range(N_CORES):
        h0 = HPC * c
        q_rows = np.r_[h0 * HD : (h0 + HPC) * HD]
        k_rows = H + q_rows
        v_rows = 2 * H + q_rows
        qk_rows = np.r_[q_rows, k_rows]
        w1t_qk = np.ascontiguousarray(w_qkv[qk_rows, :].T, dtype=np.float32)
        w1t_v = np.ascontiguousarray(w_qkv[v_rows, :].T, dtype=np.float32)
        b_qk = np.ascontiguousarray(
            b_qkv[qk_rows].reshape(O_QK // P, P).T, dtype=np.float32
        )
        b_v = np.ascontiguousarray(
            np.broadcast_to(b_qkv[v_rows], (P, O_V)), dtype=np.float32
        )
        o_lo = c * O_OUT
        wout_t = np.ascontiguousarray(
            w_out[o_lo : o_lo + O_OUT, :].T, dtype=np.float32
        )
        b_o = np.ascontiguousarray(
            np.broadcast_to(b_out[o_lo : o_lo + O_OUT], (P, O_OUT)),
            dtype=np.float32,
        )
        in_maps.append(
            {
                "xt": x,
                "w1t_qk": w1t_qk,
                "w1t_v": w1t_v,
                "b_qk": b_qk,
                "b_v": b_v,
                "wout_t": wout_t,
                "b_out": b_o,
            }
        )
    return in_maps


_program_cache = {}


def _get_program(seq=S, mm_dt=MM_DT):
    key = (seq, mm_dt)
    if key not in _program_cache:
        _program_cache[key] = _build_program(seq, mm_dt)
    return _program_cache[key]


def run(hidden_states, w_qkv, b_qkv, w_out, b_out, trace=False, mm_dt=MM_DT):
    """Run the sharded kernel; returns (output, BassKernelResults)."""
    b, s, _ = hidden_states.shape
    nc = _get_program(s, mm_dt)
    in_maps = _make_in_maps(hidden_states, w_qkv, b_qkv, w_out, b_out)
    res = run_bass_kernel_spmd(
        nc, in_maps, list(range(N_CORES)), trace=trace
    )
    cols = np.concatenate(
        [res.results[c]["out"] for c in range(N_CORES)], axis=1
    )
    return cols.reshape(b, s, H).astype(np.float32), res


def kernel(hidden_states, w_qkv, b_qkv, w_out, b_out):
    out, _ = run(
        np.asarray(hidden_states),
        np.asarray(w_qkv),
        np.asarray(b_qkv),
        np.asarray(w_out),
        np.asarray(b_out),
    )
    return out


# revision 14
# speedup vs baseline: 2.6621x; 2.6621x over previous
"""Tensor-parallel multi-head attention for Trainium2 (8 NeuronCores).

Reference computation (fp32):
    qkv = hidden @ w_qkv.T + b_qkv            # [B,S,3H]
    q,k,v = split/heads                       # [B,NH,S,HD]
    out_h = softmax(q k^T / sqrt(HD)) v       # [B,NH,S,HD]
    out = concat_heads(out_h) @ w_out.T + b_out

Sharding (Megatron-style tensor parallel over NH=16 heads, 2 heads/core):
  - hidden (transposed, [H, B*S]) replicated to all 8 cores
  - each core: QKV projection for its 2 heads -> attention for its 2 heads
    -> unnormalized context^T [256, 4096] (softmax denominators folded in
    before the gather)
  - AllGather context^T over cores -> [2048, 4096]
  - each core computes a disjoint 256-column slice of the output projection
  - host concatenates column slices: zero host FLOPs

Layout choices keep every operand in the orientation its matmul needs, so
no on-device transposes are required anywhere:
  - QKV products are produced as qkv^T ([out_dim, token]) for Q/K by making
    the weight the stationary operand; V is produced in natural [token, d]
    layout by making the activation the stationary operand.
  - scores are produced transposed (scores^T[k, q] = K Q^T), so the
    attn@V contraction over k has k on partitions for both operands.
  - softmax over k (= partitions of scores^T): exp via ACT, per-column sums
    via a DVE accumulation over k-tiles + one ones-vector matmul
    (partition reduce), reciprocal on DVE, partition-broadcast via a K=1
    ones matmul, and one multiply on the (16x smaller) context.
"""

import sys

sys.path.insert(0, "/opt/trn_rl_repo")

import numpy as np

import concourse.bass as bass
import concourse.tile as tile
from concourse import mybir
from concourse.bass_utils import run_bass_kernel_spmd
from concourse.tile import ScopedClock

FP32 = mybir.dt.float32

B = 2
S = 2048
H = 2048
NH = 16
HD = 128
N_CORES = 8
HPC = NH // N_CORES  # heads per core = 2
T = B * S  # 4096
O_QK = 2 * HPC * HD  # 512 rows of qk^T per core (Q then K)
O_V = HPC * HD  # 256
O_OUT = H // N_CORES  # 256 output columns per core
SCALE = 1.0 / float(np.sqrt(HD))
P = 128

# matmul dtype: float32 is exact (but 4 cycles/row on the PE); float32r
# reinterprets the same bits for the PE's fast fp32 path (1 cycle/row at
# moving dim >= 256). Chosen empirically; see test.py.
MM_DT = FP32


MAX_WAITS = 1  # the pinned walrus codegen rejects >1 sync wait per inst


def _wait_limit(inst):
    return MAX_WAITS


class _TileContext(tile.TileContext):
    """Tile patched for the pinned walrus codegen's sync-wait limit.

    Any instruction carrying more than MAX_WAITS semaphore waits is split:
    preceding same-engine nops carry the excess (engines execute their
    stream in order, so the waits still all precede the instruction).
    """

    def _lower_ordered_insts(self, ordered):
        nc = self.nc
        for bb_name, insts in list(ordered.items()):
            new_insts = []
            for inst in insts:
                si = inst.sync_info
                limit = _wait_limit(inst)
                if (
                    si is not None
                    and len(si.on_wait) > limit
                    and inst.engine is not None
                ):
                    waits = list(si.on_wait)
                    while len(waits) > limit:
                        chunk, waits = waits[:limit], waits[limit:]
                        new_insts.append(
                            mybir.InstNoOp(
                                name=nc.get_next_instruction_name(),
                                sync_info=mybir.SyncInfo(
                                    on_wait=chunk, on_update=[]
                                ),
                                bass_nofuse=True,
                                engine=inst.engine,
                            )
                        )
                    inst.sync_info = mybir.SyncInfo(
                        on_wait=waits, on_update=list(si.on_update)
                    )
                new_insts.append(inst)
            ordered[bb_name] = new_insts
        return super()._lower_ordered_insts(ordered)

    def _drain_and_barrier(self, tick_clock, wait_clock):
        nc = self.nc
        probe = nc.sync.nop(nofuse=True, hint="drain_wait_probe")
        wait_clock.add_sem_waits(
            probe.ins, ScopedClock({None: tick_clock.global_clock})
        )
        si = probe.ins.sync_info
        waits = list(si.on_wait) if si is not None else []
        probe.ins.sync_info = mybir.SyncInfo(
            on_wait=[], on_update=list(si.on_update) if si else []
        )
        for w in waits:
            n = nc.sync.nop(nofuse=True, hint="drain_wait_split")
            n.ins.sync_info = mybir.SyncInfo(on_wait=[w], on_update=[])
        nc.sync.drain()
        nc.all_engine_barrier()
        assert self.sems is not None
        popped = nc._tile_sem_poison_stack.pop()
        assert popped is self._sem_poison
        nc.clear_and_free_semaphores(list(self.sems.allocated().values()))
        nc.all_engine_barrier()


def _build_program(seq=S, mm_dt=MM_DT):
    """Build the SPMD Bass program (identical on all 8 cores)."""
    t_all = B * seq
    n_ht = H // P  # 16 k-tiles over the hidden dim
    ts_w = 256  # token-slice width for the QKV stage
    n_ts = t_all // ts_w
    qs_w = 512 if seq % 512 == 0 else 256  # q-slice width in attention
    n_qs = seq // qs_w
    n_kt = seq // P  # k tiles per batch in attention
    n_tt = t_all // P
    n_dt = H // P  # d tiles of the gathered context
    tg_w = 512  # token group width in the output projection
    n_tg = t_all // tg_w

    nc = bass.Bass(
        "TRN2", target_bir_lowering=False, debug=False, num_devices=N_CORES
    )

    xt = nc.dram_tensor("xt", [H, t_all], mm_dt, kind="ExternalInput")
    w1t_qk = nc.dram_tensor("w1t_qk", [H, O_QK], mm_dt, kind="ExternalInput")
    w1t_v = nc.dram_tensor("w1t_v", [H, O_V], mm_dt, kind="ExternalInput")
    b_qk = nc.dram_tensor("b_qk", [P, O_QK // P], FP32, kind="ExternalInput")
    b_v = nc.dram_tensor("b_v", [P, O_V], FP32, kind="ExternalInput")
    wout_t = nc.dram_tensor("wout_t", [H, O_OUT], mm_dt, kind="ExternalInput")
    b_out = nc.dram_tensor("b_out", [P, O_OUT], FP32, kind="ExternalInput")
    out = nc.dram_tensor("out", [t_all, O_OUT], FP32, kind="ExternalOutput")

    cc_in = nc.dram_tensor("cc_in", [O_V, t_all], mm_dt)
    cc_out = nc.dram_tensor("cc_out", [H, t_all], mm_dt, addr_space="Shared")

    xt_r = xt.ap().rearrange("(kt p) t -> p kt t", p=P)
    w1t_qk_r = w1t_qk.ap().rearrange("(kt p) o -> p kt o", p=P)
    w1t_v_r = w1t_v.ap().rearrange("(kt p) o -> p kt o", p=P)
    wout_r = wout_t.ap().rearrange("(dt p) o -> p dt o", p=P)
    cc_in_r = cc_in.ap().rearrange("(h p) t -> p h t", p=P)
    cc_out_r = cc_out.ap().rearrange("(dt p) t -> p dt t", p=P)
    out_r = out.ap().rearrange("(tt p) o -> p tt o", p=P)

    def MM(out_ap, lhsT, rhs, **kw):
        nc.tensor.matmul(out_ap, lhsT, rhs, **kw)

    with _TileContext(nc) as tc:
        with tc.tile_pool(name="const", bufs=1) as const:
            b_qk_sb = const.tile([P, O_QK // P], FP32)
            nc.sync.dma_start(b_qk_sb[:], b_qk.ap())
            b_v_sb = const.tile([P, O_V], FP32)
            nc.sync.dma_start(b_v_sb[:], b_v.ap())
            b_out_sb = const.tile([P, O_OUT], FP32)
            nc.sync.dma_start(b_out_sb[:], b_out.ap())
            ones_col = const.tile([P, 1], FP32)
            nc.vector.memset(ones_col[:], 1.0)
            ones_row = const.tile([1, P], FP32)
            nc.vector.memset(ones_row[:], 1.0)

            # ---------------- Stages 1+2 (qk^T and V resident) ----------
            acts_scope = tc.tile_pool(name="acts", bufs=1)
            acts = acts_scope.__enter__()
            qk_sb = acts.tile([P, O_QK // P, t_all], mm_dt)  # qk^T
            v_sb = acts.tile([P, n_tt, O_V], mm_dt)  # V natural

            # ---------------- Stage 1: QKV projection ----------------
            with (
                tc.tile_pool(name="wq", bufs=1) as wq,
                tc.tile_pool(name="xts", bufs=3) as xts,
                tc.tile_pool(name="ps1", bufs=2, space="PSUM") as ps1,
                tc.tile_pool(name="ps1v", bufs=2, space="PSUM") as ps1v,
            ):
                w_qk_sb = wq.tile([P, n_ht, O_QK], mm_dt)
                nc.sync.dma_start(w_qk_sb[:], w1t_qk_r)
                w_v_sb = wq.tile([P, n_ht, O_V], mm_dt)
                nc.sync.dma_start(w_v_sb[:], w1t_v_r)

                for ts_i in range(n_ts):
                    xt_t = xts.tile([P, n_ht, ts_w], mm_dt)
                    nc.sync.dma_start(
                        xt_t[:], xt_r[:, :, ts_i * ts_w : (ts_i + 1) * ts_w]
                    )
                    for ot in range(O_QK // P):
                        ps = ps1.tile([P, ts_w], FP32)
                        for kt in range(n_ht):
                            MM(
                                ps[:],
                                w_qk_sb[:, kt, ot * P : (ot + 1) * P],
                                xt_t[:, kt, :],
                                start=(kt == 0),
                                stop=(kt == n_ht - 1),
                            )
                        nc.scalar.activation(
                            qk_sb[:, ot, ts_i * ts_w : (ts_i + 1) * ts_w],
                            ps[:],
                            mybir.ActivationFunctionType.Identity,
                            bias=b_qk_sb[:, ot : ot + 1],
                        )
                    for tt in range(ts_w // P):
                        psv = ps1v.tile([P, O_V], FP32)
                        for kt in range(n_ht):
                            MM(
                                psv[:],
                                xt_t[:, kt, tt * P : (tt + 1) * P],
                                w_v_sb[:, kt, :],
                                start=(kt == 0),
                                stop=(kt == n_ht - 1),
                            )
                        nc.vector.tensor_add(
                            v_sb[:, ts_i * (ts_w // P) + tt, :], psv[:], b_v_sb[:]
                        )

            # ---------------- Stage 2: attention (2 heads) ----------------
            with (
                tc.tile_pool(name="ctxp", bufs=1) as ctxp,
                tc.tile_pool(name="exps", bufs=4) as exps,
                tc.tile_pool(name="accs", bufs=2) as accs,
                tc.tile_pool(name="sums", bufs=2) as sums,
                tc.tile_pool(name="ps_s", bufs=2, space="PSUM") as ps_s_pool,
                tc.tile_pool(name="ps_c", bufs=2, space="PSUM") as ps_c_pool,
                tc.tile_pool(name="ps_r", bufs=2, space="PSUM") as ps_r_pool,
            ):
                ctx_sb = ctxp.tile([P, HPC, t_all], mm_dt)  # context^T
                for b in range(B):
                    for h in range(HPC):
                        for qs in range(n_qs):
                            q_lo = b * seq + qs * qs_w
                            ps_ctx = ps_c_pool.tile([P, qs_w], FP32)
                            acc = accs.tile([P, qs_w], FP32)
                            for kt in range(n_kt):
                                k_lo = b * seq + kt * P
                                ps_sc = ps_s_pool.tile([P, qs_w], FP32)
                                MM(
                                    ps_sc[:],
                                    qk_sb[:, HPC + h, k_lo : k_lo + P],
                                    qk_sb[:, h, q_lo : q_lo + qs_w],
                                    start=True,
                                    stop=True,
                                )
                                exp_t = exps.tile([P, qs_w], mm_dt)
                                nc.scalar.activation(
                                    exp_t[:],
                                    ps_sc[:],
                                    mybir.ActivationFunctionType.Exp,
                                    scale=SCALE,
                                )
                                if kt == 0:
                                    nc.vector.tensor_copy(acc[:], exp_t[:])
                                else:
                                    nc.vector.tensor_add(acc[:], acc[:], exp_t[:])
                                MM(
                                    ps_ctx[:],
                                    v_sb[
                                        :,
                                        (b * seq) // P + kt,
                                        h * HD : (h + 1) * HD,
                                    ],
                                    exp_t[:],
                                    start=(kt == 0),
                                    stop=(kt == n_kt - 1),
                                )
                            # softmax denominators: partition-reduce acc,
                            # reciprocal, broadcast back over partitions
                            ps_sum = ps_r_pool.tile([1, qs_w], FP32)
                            MM(
                                ps_sum[:], ones_col[:], acc[:], start=True, stop=True
                            )
                            inv = sums.tile([1, qs_w], FP32)
                            nc.vector.reciprocal(inv[:], ps_sum[:])
                            ps_b = ps_r_pool.tile([P, qs_w], FP32)
                            MM(
                                ps_b[:], ones_row[:], inv[:], start=True, stop=True
                            )
                            invb = sums.tile([P, qs_w], FP32)
                            nc.vector.tensor_copy(invb[:], ps_b[:])
                            nc.vector.tensor_mul(
                                ctx_sb[:, h, q_lo : q_lo + qs_w],
                                ps_ctx[:],
                                invb[:],
                            )

                # stash context^T to DRAM for the gather (inside the scope
                # so the pool free waits on this read)
                nc.sync.dma_start(cc_in_r, ctx_sb[:])

            # release qk^T / V buffers before stage 3 allocates
            acts_scope.__exit__(None, None, None)

            # gather context across cores
            nc.gpsimd.collective_compute(
                "AllGather",
                mybir.AluOpType.bypass,
                replica_groups=[list(range(N_CORES))],
                ins=[cc_in.ap()],
                outs=[cc_out.ap()],
            )

            # ---------------- Stage 3: output projection ----------------
            with (
                tc.tile_pool(name="wo", bufs=1) as wo,
                tc.tile_pool(name="ctxs", bufs=3) as ctxs,
                tc.tile_pool(name="outs", bufs=3) as outs,
                tc.tile_pool(name="ps3", bufs=2, space="PSUM") as ps3,
            ):
                wout_sb = wo.tile([P, n_dt, O_OUT], mm_dt)
                nc.sync.dma_start(wout_sb[:], wout_r)
                for tg in range(n_tg):
                    ctx_t = ctxs.tile([P, n_dt, tg_w], mm_dt)
                    nc.sync.dma_start(
                        ctx_t[:], cc_out_r[:, :, tg * tg_w : (tg + 1) * tg_w]
                    )
                    for tt in range(tg_w // P):
                        ps_o = ps3.tile([P, O_OUT], FP32)
                        for dt in range(n_dt):
                            MM(
                                ps_o[:],
                                ctx_t[:, dt, tt * P : (tt + 1) * P],
                                wout_sb[:, dt, :],
                                start=(dt == 0),
                                stop=(dt == n_dt - 1),
                            )
                        out_t = outs.tile([P, O_OUT], FP32)
                        nc.vector.tensor_add(out_t[:], ps_o[:], b_out_sb[:])
                        nc.sync.dma_start(
                            out_r[:, tg * (tg_w // P) + tt, :], out_t[:]
                        )

    return nc


def _make_in_maps(hidden_states, w_qkv, b_qkv, w_out, b_out):
    b, s, _ = hidden_states.shape
    t_all = b * s
    x = np.ascontiguousarray(
        hidden_states.reshape(t_all, H).T, dtype=np.float32
    )  # [H, T]
    in_maps = []
    for c in range(N_CORES):
        h0 = HPC * c
        q_rows = np.r_[h0 * HD : (h0 + HPC) * HD]
        k_rows = H + q_rows
        v_rows = 2 * H + q_rows
        qk_rows = np.r_[q_rows, k_rows]
        w1t_qk = np.ascontiguousarray(w_qkv[qk_rows, :].T, dtype=np.float32)
        w1t_v = np.ascontiguousarray(w_qkv[v_rows, :].T, dtype=np.float32)
        b_qk = np.ascontiguousarray(
            b_qkv[qk_rows].reshape(O_QK // P, P).T, dtype=np.float32
        )
        b_v = np.ascontiguousarray(
            np.broadcast_to(b_qkv[v_rows], (P, O_V)), dtype=np.float32
        )
        o_lo = c * O_OUT
        wout_t = np.ascontiguousarray(
            w_out[o_lo : o_lo + O_OUT, :].T, dtype=np.float32
        )
        b_o = np.ascontiguousarray(
            np.broadcast_to(b_out[o_lo : o_lo + O_OUT], (P, O_OUT)),
            dtype=np.float32,
        )
        in_maps.append(
            {
                "xt": x,
                "w1t_qk": w1t_qk,
                "w1t_v": w1t_v,
                "b_qk": b_qk,
                "b_v": b_v,
                "wout_t": wout_t,
                "b_out": b_o,
            }
        )
    return in_maps


_program_cache = {}


def _get_program(seq=S, mm_dt=MM_DT):
    key = (seq, mm_dt)
    if key not in _program_cache:
        _program_cache[key] = _build_program(seq, mm_dt)
    return _program_cache[key]


def run(hidden_states, w_qkv, b_qkv, w_out, b_out, trace=False, mm_dt=MM_DT):
    """Run the sharded kernel; returns (output, BassKernelResults)."""
    b, s, _ = hidden_states.shape
    nc = _get_program(s, mm_dt)
    in_maps = _make_in_maps(hidden_states, w_qkv, b_qkv, w_out, b_out)
    res = run_bass_kernel_spmd(
        nc, in_maps, list(range(N_CORES)), trace=trace
    )
    cols = np.concatenate(
        [res.results[c]["out"] for c in range(N_CORES)], axis=1
    )
    return cols.reshape(b, s, H).astype(np.float32), res


def kernel(hidden_states, w_qkv, b_qkv, w_out, b_out):
    out, _ = run(
        np.asarray(hidden_states),
        np.asarray(w_qkv),
        np.asarray(b_qkv),
        np.asarray(w_out),
        np.asarray(b_out),
    )
    return out


# revision 19
# speedup vs baseline: 3.0057x; 1.1291x over previous
"""Tensor-parallel multi-head attention for Trainium2 (8 NeuronCores).

Reference computation (fp32):
    qkv = hidden @ w_qkv.T + b_qkv            # [B,S,3H]
    q,k,v = split/heads                       # [B,NH,S,HD]
    out_h = softmax(q k^T / sqrt(HD)) v       # [B,NH,S,HD]
    out = concat_heads(out_h) @ w_out.T + b_out

Sharding (Megatron-style tensor parallel over NH=16 heads, 2 heads/core):
  - hidden (transposed, [H, B*S]) replicated to all 8 cores
  - each core: QKV projection for its 2 heads -> attention for its 2 heads
    -> unnormalized context^T [256, 4096] (softmax denominators folded in
    before the gather)
  - AllGather context^T over cores -> [2048, 4096]
  - each core computes a disjoint 256-column slice of the output projection
  - host concatenates column slices: zero host FLOPs

Layout choices keep every operand in the orientation its matmul needs, so
no on-device transposes are required anywhere:
  - QKV products are produced as qkv^T ([out_dim, token]) for Q/K by making
    the weight the stationary operand; V is produced in natural [token, d]
    layout by making the activation the stationary operand.
  - scores are produced transposed (scores^T[k, q] = K Q^T), so the
    attn@V contraction over k has k on partitions for both operands.
  - softmax over k (= partitions of scores^T): exp via ACT, per-column sums
    via a DVE accumulation over k-tiles + one ones-vector matmul
    (partition reduce), reciprocal on DVE, partition-broadcast via a K=1
    ones matmul, and one multiply on the (16x smaller) context.
"""

import sys

sys.path.insert(0, "/opt/trn_rl_repo")

import numpy as np

import concourse.bass as bass
import concourse.tile as tile
from concourse import mybir
from concourse.bass_utils import run_bass_kernel_spmd
from concourse.tile import ScopedClock

FP32 = mybir.dt.float32

B = 2
S = 2048
H = 2048
NH = 16
HD = 128
N_CORES = 8
HPC = NH // N_CORES  # heads per core = 2
T = B * S  # 4096
O_QK = 2 * HPC * HD  # 512 rows of qk^T per core (Q then K)
O_V = HPC * HD  # 256
O_OUT = H // N_CORES  # 256 output columns per core
SCALE = 1.0 / float(np.sqrt(HD))
P = 128

# matmul dtype: float32 is exact (but 4 cycles/row on the PE); float32r
# reinterprets the same bits for the PE's fast fp32 path (1 cycle/row at
# moving dim >= 256). Chosen empirically; see test.py.
MM_DT = FP32


MAX_WAITS = 1  # the pinned walrus codegen rejects >1 sync wait per inst


def _wait_limit(inst):
    return MAX_WAITS


class _TileContext(tile.TileContext):
    """Tile patched for the pinned walrus codegen's sync-wait limit.

    Any instruction carrying more than MAX_WAITS semaphore waits is split:
    preceding same-engine nops carry the excess (engines execute their
    stream in order, so the waits still all precede the instruction).
    """

    def _lower_ordered_insts(self, ordered):
        nc = self.nc
        for bb_name, insts in list(ordered.items()):
            new_insts = []
            for inst in insts:
                si = inst.sync_info
                limit = _wait_limit(inst)
                if (
                    si is not None
                    and len(si.on_wait) > limit
                    and inst.engine is not None
                ):
                    waits = list(si.on_wait)
                    while len(waits) > limit:
                        chunk, waits = waits[:limit], waits[limit:]
                        new_insts.append(
                            mybir.InstNoOp(
                                name=nc.get_next_instruction_name(),
                                sync_info=mybir.SyncInfo(
                                    on_wait=chunk, on_update=[]
                                ),
                                bass_nofuse=True,
                                engine=inst.engine,
                            )
                        )
                    inst.sync_info = mybir.SyncInfo(
                        on_wait=waits, on_update=list(si.on_update)
                    )
                new_insts.append(inst)
            ordered[bb_name] = new_insts
        return super()._lower_ordered_insts(ordered)

    def _drain_and_barrier(self, tick_clock, wait_clock):
        nc = self.nc
        probe = nc.sync.nop(nofuse=True, hint="drain_wait_probe")
        wait_clock.add_sem_waits(
            probe.ins, ScopedClock({None: tick_clock.global_clock})
        )
        si = probe.ins.sync_info
        waits = list(si.on_wait) if si is not None else []
        probe.ins.sync_info = mybir.SyncInfo(
            on_wait=[], on_update=list(si.on_update) if si else []
        )
        for w in waits:
            n = nc.sync.nop(nofuse=True, hint="drain_wait_split")
            n.ins.sync_info = mybir.SyncInfo(on_wait=[w], on_update=[])
        nc.sync.drain()
        nc.all_engine_barrier()
        assert self.sems is not None
        popped = nc._tile_sem_poison_stack.pop()
        assert popped is self._sem_poison
        nc.clear_and_free_semaphores(list(self.sems.allocated().values()))
        nc.all_engine_barrier()


def _build_program(seq=S, mm_dt=MM_DT):
    """Build the SPMD Bass program (identical on all 8 cores)."""
    t_all = B * seq
    n_ht = H // P  # 16 k-tiles over the hidden dim
    ts_w = 256  # token-slice width for the QKV stage
    n_ts = t_all // ts_w
    qs_w = 512 if seq % 512 == 0 else 256  # q-slice width in attention
    n_qs = seq // qs_w
    n_kt = seq // P  # k tiles per batch in attention
    n_tt = t_all // P
    n_dt = H // P  # d tiles of the gathered context
    tg_w = 512  # token group width in the output projection
    n_tg = t_all // tg_w

    nc = bass.Bass(
        "TRN2", target_bir_lowering=False, debug=False, num_devices=N_CORES
    )

    xt = nc.dram_tensor("xt", [H, t_all], mm_dt, kind="ExternalInput")
    w1t_qk = nc.dram_tensor("w1t_qk", [H, O_QK], mm_dt, kind="ExternalInput")
    w1t_v = nc.dram_tensor("w1t_v", [H, O_V], mm_dt, kind="ExternalInput")
    b_qk = nc.dram_tensor("b_qk", [P, O_QK // P], FP32, kind="ExternalInput")
    b_v = nc.dram_tensor("b_v", [P, O_V], FP32, kind="ExternalInput")
    wout_t = nc.dram_tensor("wout_t", [H, O_OUT], mm_dt, kind="ExternalInput")
    b_out = nc.dram_tensor("b_out", [P, O_OUT // P], FP32, kind="ExternalInput")
    ones_d = nc.dram_tensor("ones_d", [P, 1], mm_dt, kind="ExternalInput")
    out = nc.dram_tensor("out", [O_OUT, t_all], FP32, kind="ExternalOutput")

    n_ch = B * n_qs  # token chunks, gathered + projected as they finish
    cc_in = nc.dram_tensor("cc_in", [n_ch, O_V, qs_w], mm_dt)
    cc_out = nc.dram_tensor("cc_out", [n_ch, H, qs_w], mm_dt, addr_space="Shared")

    xt_r = xt.ap().rearrange("(kt p) t -> p kt t", p=P)
    w1t_qk_r = w1t_qk.ap().rearrange("(kt p) o -> p kt o", p=P)
    w1t_v_r = w1t_v.ap().rearrange("(kt p) o -> p kt o", p=P)
    wout_r = wout_t.ap().rearrange("(dt p) o -> p dt o", p=P)
    cc_in_r = cc_in.ap().rearrange("c (h p) t -> c p h t", p=P)
    cc_out_r = cc_out.ap().rearrange("c (dt p) t -> c p dt t", p=P)
    out_r = out.ap().rearrange("(ot p) t -> p ot t", p=P)

    def MM(out_ap, lhsT, rhs, **kw):
        nc.tensor.matmul(out_ap, lhsT, rhs, **kw)

    with _TileContext(nc) as tc:
        with tc.tile_pool(name="const", bufs=1) as const:
            b_qk_sb = const.tile([P, O_QK // P], FP32)
            nc.sync.dma_start(b_qk_sb[:], b_qk.ap())
            b_v_sb = const.tile([P, O_V], FP32)
            nc.sync.dma_start(b_v_sb[:], b_v.ap())
            b_out_sb = const.tile([P, O_OUT // P], FP32)
            nc.sync.dma_start(b_out_sb[:], b_out.ap())
            ones_col = const.tile([P, 1], mm_dt)
            nc.sync.dma_start(ones_col[:], ones_d.ap())
            ones_row = const.tile([1, P], FP32)
            nc.vector.memset(ones_row[:], 1.0)

            # ---------------- Stages 1+2 (qk^T and V resident) ----------
            acts_scope = tc.tile_pool(name="acts", bufs=1)
            acts = acts_scope.__enter__()
            qk_sb = acts.tile([P, O_QK // P, t_all], mm_dt)  # qk^T
            v_sb = acts.tile([P, n_tt, O_V], mm_dt)  # V natural

            # ---------------- Stage 1: QKV projection ----------------
            with (
                tc.tile_pool(name="wq", bufs=1) as wq,
                tc.tile_pool(name="xts", bufs=3) as xts,
                tc.tile_pool(name="ps1", bufs=2, space="PSUM") as ps1,
                tc.tile_pool(name="ps1v", bufs=2, space="PSUM") as ps1v,
            ):
                w_qk_sb = wq.tile([P, n_ht, O_QK], mm_dt)
                nc.sync.dma_start(w_qk_sb[:], w1t_qk_r)
                w_v_sb = wq.tile([P, n_ht, O_V], mm_dt)
                nc.sync.dma_start(w_v_sb[:], w1t_v_r)

                for ts_i in range(n_ts):
                    xt_t = xts.tile([P, n_ht, ts_w], mm_dt)
                    nc.sync.dma_start(
                        xt_t[:], xt_r[:, :, ts_i * ts_w : (ts_i + 1) * ts_w]
                    )
                    for ot in range(O_QK // P):
                        ps = ps1.tile([P, ts_w], FP32)
                        for kt in range(n_ht):
                            MM(
                                ps[:],
                                w_qk_sb[:, kt, ot * P : (ot + 1) * P],
                                xt_t[:, kt, :],
                                start=(kt == 0),
                                stop=(kt == n_ht - 1),
                            )
                        nc.scalar.activation(
                            qk_sb[:, ot, ts_i * ts_w : (ts_i + 1) * ts_w],
                            ps[:],
                            mybir.ActivationFunctionType.Identity,
                            bias=b_qk_sb[:, ot : ot + 1],
                        )
                    for tt in range(ts_w // P):
                        psv = ps1v.tile([P, O_V], FP32)
                        for kt in range(n_ht):
                            MM(
                                psv[:],
                                xt_t[:, kt, tt * P : (tt + 1) * P],
                                w_v_sb[:, kt, :],
                                start=(kt == 0),
                                stop=(kt == n_ht - 1),
                            )
                        nc.vector.tensor_add(
                            v_sb[:, ts_i * (ts_w // P) + tt, :], psv[:], b_v_sb[:]
                        )

            # ------- Stages 2+3 fused: attention -> gather -> projection ----
            # per 512-token chunk: attention for both heads, ship ctx^T via
            # a chunk AllGather, and run that chunk's output projection --
            # collectives and stage-3 DMA overlap later chunks' attention.
            with (
                tc.tile_pool(name="wo", bufs=1) as wo,
                tc.tile_pool(name="ctxp", bufs=3) as ctxp,
                tc.tile_pool(name="exps", bufs=4) as exps,
                tc.tile_pool(name="sums", bufs=2) as sums,
                tc.tile_pool(name="ctxs", bufs=2) as ctxs,
                tc.tile_pool(name="outs", bufs=3) as outs,
                tc.tile_pool(name="ps_s", bufs=2, space="PSUM") as ps_s_pool,
                tc.tile_pool(name="ps_c", bufs=2, space="PSUM") as ps_c_pool,
                tc.tile_pool(name="ps_r", bufs=1, space="PSUM") as ps_r_pool,
                tc.tile_pool(name="ps3", bufs=2, space="PSUM") as ps3,
            ):
                wout_sb = wo.tile([P, n_dt, O_OUT], mm_dt)
                nc.sync.dma_start(wout_sb[:], wout_r)
                sub_w = 256  # stage-3 token sub-chunk (DMA/SBUF granularity)
                for ch in range(n_ch):
                    b, qs = divmod(ch, n_qs)
                    q_lo = b * seq + qs * qs_w
                    ctx_ch = ctxp.tile([P, HPC, qs_w], mm_dt)
                    for h in range(HPC):
                        ps_ctx = ps_c_pool.tile([P, qs_w], FP32)
                        ps_sum = ps_r_pool.tile([1, qs_w], FP32)
                        for kt in range(n_kt):
                            k_lo = b * seq + kt * P
                            ps_sc = ps_s_pool.tile([P, qs_w], FP32)
                            MM(
                                ps_sc[:],
                                qk_sb[:, HPC + h, k_lo : k_lo + P],
                                qk_sb[:, h, q_lo : q_lo + qs_w],
                                start=True,
                                stop=True,
                            )
                            exp_t = exps.tile([P, qs_w], mm_dt)
                            nc.scalar.activation(
                                exp_t[:],
                                ps_sc[:],
                                mybir.ActivationFunctionType.Exp,
                                scale=SCALE,
                            )
                            MM(
                                ps_ctx[:],
                                v_sb[:, (b * seq) // P + kt, h * HD : (h + 1) * HD],
                                exp_t[:],
                                start=(kt == 0),
                                stop=(kt == n_kt - 1),
                            )
                            # denominator: accumulate column sums on the PE
                            MM(
                                ps_sum[:],
                                ones_col[:],
                                exp_t[:],
                                start=(kt == 0),
                                stop=(kt == n_kt - 1),
                            )
                        inv = sums.tile([1, qs_w], FP32)
                        nc.vector.reciprocal(inv[:], ps_sum[:])
                        ps_b = ps_r_pool.tile([P, qs_w], FP32)
                        nc.tensor.matmul(
                            ps_b[:], ones_row[:], inv[:], start=True, stop=True
                        )
                        invb = sums.tile([P, qs_w], FP32)
                        nc.vector.tensor_copy(invb[:], ps_b[:])
                        nc.vector.tensor_mul(
                            ctx_ch[:, h, :], ps_ctx[:], invb[:]
                        )
                    # ship this chunk's context and gather across cores
                    nc.sync.dma_start(cc_in_r[ch], ctx_ch[:])
                    nc.gpsimd.collective_compute(
                        "AllGather",
                        mybir.AluOpType.bypass,
                        replica_groups=[list(range(N_CORES))],
                        ins=[cc_in.ap()[ch]],
                        outs=[cc_out.ap()[ch]],
                    )
                    # output projection for this chunk (out^T: o on partitions)
                    for sub in range(qs_w // sub_w):
                        t_lo = q_lo + sub * sub_w
                        ctx_t = ctxs.tile([P, n_dt, sub_w], mm_dt)
                        nc.sync.dma_start(
                            ctx_t[:],
                            cc_out_r[ch][:, :, sub * sub_w : (sub + 1) * sub_w],
                        )
                        for ot in range(O_OUT // P):
                            ps_o = ps3.tile([P, sub_w], FP32)
                            for dt in range(n_dt):
                                MM(
                                    ps_o[:],
                                    wout_sb[:, dt, ot * P : (ot + 1) * P],
                                    ctx_t[:, dt, :],
                                    start=(dt == 0),
                                    stop=(dt == n_dt - 1),
                                )
                            out_t = outs.tile([P, sub_w], FP32)
                            nc.scalar.activation(
                                out_t[:],
                                ps_o[:],
                                mybir.ActivationFunctionType.Identity,
                                bias=b_out_sb[:, ot : ot + 1],
                            )
                            nc.sync.dma_start(
                                out_r[:, ot, t_lo : t_lo + sub_w], out_t[:]
                            )

            acts_scope.__exit__(None, None, None)

    return nc


def _make_in_maps(hidden_states, w_qkv, b_qkv, w_out, b_out):
    b, s, _ = hidden_states.shape
    t_all = b * s
    x = np.ascontiguousarray(
        hidden_states.reshape(t_all, H).T, dtype=np.float32
    )  # [H, T]
    in_maps = []
    for c in range(N_CORES):
        h0 = HPC * c
        q_rows = np.r_[h0 * HD : (h0 + HPC) * HD]
        k_rows = H + q_rows
        v_rows = 2 * H + q_rows
        qk_rows = np.r_[q_rows, k_rows]
        w1t_qk = np.ascontiguousarray(w_qkv[qk_rows, :].T, dtype=np.float32)
        w1t_v = np.ascontiguousarray(w_qkv[v_rows, :].T, dtype=np.float32)
        b_qk = np.ascontiguousarray(
            b_qkv[qk_rows].reshape(O_QK // P, P).T, dtype=np.float32
        )
        b_v = np.ascontiguousarray(
            np.broadcast_to(b_qkv[v_rows], (P, O_V)), dtype=np.float32
        )
        o_lo = c * O_OUT
        wout_t = np.ascontiguousarray(
            w_out[o_lo : o_lo + O_OUT, :].T, dtype=np.float32
        )
        b_o = np.ascontiguousarray(
            b_out[o_lo : o_lo + O_OUT].reshape(O_OUT // P, P).T,
            dtype=np.float32,
        )
        in_maps.append(
            {
                "ones_d": np.ones((P, 1), dtype=np.float32),
                "xt": x,
                "w1t_qk": w1t_qk,
                "w1t_v": w1t_v,
                "b_qk": b_qk,
                "b_v": b_v,
                "wout_t": wout_t,
                "b_out": b_o,
            }
        )
    return in_maps


_program_cache = {}


def _get_program(seq=S, mm_dt=MM_DT):
    key = (seq, mm_dt)
    if key not in _program_cache:
        _program_cache[key] = _build_program(seq, mm_dt)
    return _program_cache[key]


def run(hidden_states, w_qkv, b_qkv, w_out, b_out, trace=False, mm_dt=MM_DT):
    """Run the sharded kernel; returns (output, BassKernelResults)."""
    b, s, _ = hidden_states.shape
    nc = _get_program(s, mm_dt)
    in_maps = _make_in_maps(hidden_states, w_qkv, b_qkv, w_out, b_out)
    res = run_bass_kernel_spmd(
        nc, in_maps, list(range(N_CORES)), trace=trace
    )
    # per-core output is out^T [O_OUT, T]; stack to [H, T] then transpose
    cols = np.concatenate([res.results[c]["out"] for c in range(N_CORES)], axis=0)
    return (
        np.ascontiguousarray(cols.T).reshape(b, s, H).astype(np.float32),
        res,
    )


def kernel(hidden_states, w_qkv, b_qkv, w_out, b_out):
    out, _ = run(
        np.asarray(hidden_states),
        np.asarray(w_qkv),
        np.asarray(b_qkv),
        np.asarray(w_out),
        np.asarray(b_out),
    )
    return out


# revision 20
# speedup vs baseline: 3.4448x; 1.1461x over previous
"""Tensor-parallel multi-head attention for Trainium2 (8 NeuronCores).

Reference computation (fp32):
    qkv = hidden @ w_qkv.T + b_qkv            # [B,S,3H]
    q,k,v = split/heads                       # [B,NH,S,HD]
    out_h = softmax(q k^T / sqrt(HD)) v       # [B,NH,S,HD]
    out = concat_heads(out_h) @ w_out.T + b_out

Sharding (Megatron-style tensor parallel over NH=16 heads, 2 heads/core):
  - hidden (transposed, [H, B*S]) replicated to all 8 cores
  - each core: QKV projection for its 2 heads -> attention for its 2 heads
    -> unnormalized context^T [256, 4096] (softmax denominators folded in
    before the gather)
  - AllGather context^T over cores -> [2048, 4096]
  - each core computes a disjoint 256-column slice of the output projection
  - host concatenates column slices: zero host FLOPs

Layout choices keep every operand in the orientation its matmul needs, so
no on-device transposes are required anywhere:
  - QKV products are produced as qkv^T ([out_dim, token]) for Q/K by making
    the weight the stationary operand; V is produced in natural [token, d]
    layout by making the activation the stationary operand.
  - scores are produced transposed (scores^T[k, q] = K Q^T), so the
    attn@V contraction over k has k on partitions for both operands.
  - softmax over k (= partitions of scores^T): exp via ACT, per-column sums
    via a DVE accumulation over k-tiles + one ones-vector matmul
    (partition reduce), reciprocal on DVE, partition-broadcast via a K=1
    ones matmul, and one multiply on the (16x smaller) context.
"""

import sys

sys.path.insert(0, "/opt/trn_rl_repo")

import numpy as np

import concourse.bass as bass
import concourse.tile as tile
from concourse import mybir
from concourse.bass_utils import run_bass_kernel_spmd
from concourse.tile import ScopedClock

FP32 = mybir.dt.float32

B = 2
S = 2048
H = 2048
NH = 16
HD = 128
N_CORES = 8
HPC = NH // N_CORES  # heads per core = 2
T = B * S  # 4096
O_QK = 2 * HPC * HD  # 512 rows of qk^T per core (Q then K)
O_V = HPC * HD  # 256
O_OUT = H // N_CORES  # 256 output columns per core
SCALE = 1.0 / float(np.sqrt(HD))
P = 128

# matmul dtype: float32 is exact (but 4 cycles/row on the PE); float32r
# reinterprets the same bits for the PE's fast fp32 path (1 cycle/row at
# moving dim >= 256). Chosen empirically; see test.py.
MM_DT = FP32


MAX_WAITS = 1  # the pinned walrus codegen rejects >1 sync wait per inst


def _wait_limit(inst):
    return MAX_WAITS


class _TileContext(tile.TileContext):
    """Tile patched for the pinned walrus codegen's sync-wait limit.

    Any instruction carrying more than MAX_WAITS semaphore waits is split:
    preceding same-engine nops carry the excess (engines execute their
    stream in order, so the waits still all precede the instruction).
    """

    def _lower_ordered_insts(self, ordered):
        nc = self.nc
        for bb_name, insts in list(ordered.items()):
            new_insts = []
            for inst in insts:
                si = inst.sync_info
                limit = _wait_limit(inst)
                if (
                    si is not None
                    and len(si.on_wait) > limit
                    and inst.engine is not None
                ):
                    waits = list(si.on_wait)
                    while len(waits) > limit:
                        chunk, waits = waits[:limit], waits[limit:]
                        new_insts.append(
                            mybir.InstNoOp(
                                name=nc.get_next_instruction_name(),
                                sync_info=mybir.SyncInfo(
                                    on_wait=chunk, on_update=[]
                                ),
                                bass_nofuse=True,
                                engine=inst.engine,
                            )
                        )
                    inst.sync_info = mybir.SyncInfo(
                        on_wait=waits, on_update=list(si.on_update)
                    )
                new_insts.append(inst)
            ordered[bb_name] = new_insts
        return super()._lower_ordered_insts(ordered)

    def _drain_and_barrier(self, tick_clock, wait_clock):
        nc = self.nc
        probe = nc.sync.nop(nofuse=True, hint="drain_wait_probe")
        wait_clock.add_sem_waits(
            probe.ins, ScopedClock({None: tick_clock.global_clock})
        )
        si = probe.ins.sync_info
        waits = list(si.on_wait) if si is not None else []
        probe.ins.sync_info = mybir.SyncInfo(
            on_wait=[], on_update=list(si.on_update) if si else []
        )
        for w in waits:
            n = nc.sync.nop(nofuse=True, hint="drain_wait_split")
            n.ins.sync_info = mybir.SyncInfo(on_wait=[w], on_update=[])
        nc.sync.drain()
        nc.all_engine_barrier()
        assert self.sems is not None
        popped = nc._tile_sem_poison_stack.pop()
        assert popped is self._sem_poison
        nc.clear_and_free_semaphores(list(self.sems.allocated().values()))
        nc.all_engine_barrier()


def _build_program(seq=S, mm_dt=MM_DT):
    """Build the SPMD Bass program (identical on all 8 cores)."""
    t_all = B * seq
    n_ht = H // P  # 16 k-tiles over the hidden dim
    ts_w = 256  # token-slice width for the QKV stage
    n_ts = t_all // ts_w
    qs_w = 512 if seq % 512 == 0 else 256  # q-slice width in attention
    n_qs = seq // qs_w
    n_kt = seq // P  # k tiles per batch in attention
    n_tt = t_all // P
    n_dt = H // P  # d tiles of the gathered context
    tg_w = 512  # token group width in the output projection
    n_tg = t_all // tg_w

    nc = bass.Bass(
        "TRN2", target_bir_lowering=False, debug=False, num_devices=N_CORES
    )

    xt = nc.dram_tensor("xt", [H, t_all], mm_dt, kind="ExternalInput")
    w1t_qk = nc.dram_tensor("w1t_qk", [H, O_QK], mm_dt, kind="ExternalInput")
    w1t_v = nc.dram_tensor("w1t_v", [H, O_V], mm_dt, kind="ExternalInput")
    b_qk = nc.dram_tensor("b_qk", [P, O_QK // P], FP32, kind="ExternalInput")
    b_v = nc.dram_tensor("b_v", [P, O_V], FP32, kind="ExternalInput")
    wout_t = nc.dram_tensor("wout_t", [H, O_OUT], mm_dt, kind="ExternalInput")
    b_out = nc.dram_tensor("b_out", [P, O_OUT // P], FP32, kind="ExternalInput")
    ones_d = nc.dram_tensor("ones_d", [P, 1], mm_dt, kind="ExternalInput")
    out = nc.dram_tensor("out", [O_OUT, t_all], FP32, kind="ExternalOutput")

    n_ch = B * n_qs  # token chunks, gathered + projected as they finish
    cc_in = nc.dram_tensor("cc_in", [n_ch, O_V, qs_w], mm_dt)
    cc_out = nc.dram_tensor("cc_out", [n_ch, H, qs_w], mm_dt, addr_space="Shared")

    xt_r = xt.ap().rearrange("(kt p) t -> p kt t", p=P)
    w1t_qk_r = w1t_qk.ap().rearrange("(kt p) o -> p kt o", p=P)
    w1t_v_r = w1t_v.ap().rearrange("(kt p) o -> p kt o", p=P)
    wout_r = wout_t.ap().rearrange("(dt p) o -> p dt o", p=P)
    cc_in_r = cc_in.ap().rearrange("c (h p) t -> c p h t", p=P)
    cc_out_r = cc_out.ap().rearrange("c (dt p) t -> c p dt t", p=P)
    out_r = out.ap().rearrange("(ot p) t -> p ot t", p=P)

    def MM(out_ap, lhsT, rhs, **kw):
        nc.tensor.matmul(out_ap, lhsT, rhs, **kw)

    with _TileContext(nc) as tc:
        with tc.tile_pool(name="const", bufs=1) as const:
            b_qk_sb = const.tile([P, O_QK // P], FP32)
            nc.sync.dma_start(b_qk_sb[:], b_qk.ap())
            b_v_sb = const.tile([P, O_V], FP32)
            nc.sync.dma_start(b_v_sb[:], b_v.ap())
            b_out_sb = const.tile([P, O_OUT // P], FP32)
            nc.sync.dma_start(b_out_sb[:], b_out.ap())
            ones_col = const.tile([P, 1], mm_dt)
            nc.sync.dma_start(ones_col[:], ones_d.ap())
            ones_row = const.tile([1, P], FP32)
            nc.vector.memset(ones_row[:], 1.0)

            # ---------------- Stages 1+2 (qk^T and V resident) ----------
            acts_scope = tc.tile_pool(name="acts", bufs=1)
            acts = acts_scope.__enter__()
            qk_sb = acts.tile([P, O_QK // P, t_all], mm_dt)  # qk^T
            v_sb = acts.tile([P, n_tt, O_V], mm_dt)  # V natural

            # ---------------- Stage 1: QKV projection ----------------
            with (
                tc.tile_pool(name="wq", bufs=1) as wq,
                tc.tile_pool(name="xts", bufs=3) as xts,
                tc.tile_pool(name="ps1", bufs=2, space="PSUM") as ps1,
                tc.tile_pool(name="ps1v", bufs=2, space="PSUM") as ps1v,
            ):
                w_qk_sb = wq.tile([P, n_ht, O_QK], mm_dt)
                nc.sync.dma_start(w_qk_sb[:], w1t_qk_r)
                w_v_sb = wq.tile([P, n_ht, O_V], mm_dt)
                nc.sync.dma_start(w_v_sb[:], w1t_v_r)

                for ts_i in range(n_ts):
                    xt_t = xts.tile([P, n_ht, ts_w], mm_dt)
                    nc.sync.dma_start(
                        xt_t[:], xt_r[:, :, ts_i * ts_w : (ts_i + 1) * ts_w]
                    )
                    for ot in range(O_QK // P):
                        ps = ps1.tile([P, ts_w], FP32)
                        for kt in range(n_ht):
                            MM(
                                ps[:],
                                w_qk_sb[:, kt, ot * P : (ot + 1) * P],
                                xt_t[:, kt, :],
                                start=(kt == 0),
                                stop=(kt == n_ht - 1),
                            )
                        nc.scalar.activation(
                            qk_sb[:, ot, ts_i * ts_w : (ts_i + 1) * ts_w],
                            ps[:],
                            mybir.ActivationFunctionType.Identity,
                            bias=b_qk_sb[:, ot : ot + 1],
                        )
                    for tt in range(ts_w // P):
                        psv = ps1v.tile([P, O_V], FP32)
                        for kt in range(n_ht):
                            MM(
                                psv[:],
                                xt_t[:, kt, tt * P : (tt + 1) * P],
                                w_v_sb[:, kt, :],
                                start=(kt == 0),
                                stop=(kt == n_ht - 1),
                            )
                        nc.vector.tensor_add(
                            v_sb[:, ts_i * (ts_w // P) + tt, :], psv[:], b_v_sb[:]
                        )

            # ------- Stages 2+3 fused: attention -> gather -> projection ----
            # per 512-token chunk: attention for both heads, ship ctx^T via
            # a chunk AllGather, and run that chunk's output projection --
            # collectives and stage-3 DMA overlap later chunks' attention.
            with (
                tc.tile_pool(name="wo", bufs=1) as wo,
                tc.tile_pool(name="ctxp", bufs=3) as ctxp,
                tc.tile_pool(name="exps", bufs=4) as exps,
                tc.tile_pool(name="sums", bufs=2) as sums,
                tc.tile_pool(name="ctxs", bufs=2) as ctxs,
                tc.tile_pool(name="outs", bufs=3) as outs,
                tc.tile_pool(name="ps_s", bufs=2, space="PSUM") as ps_s_pool,
                tc.tile_pool(name="ps_c", bufs=2, space="PSUM") as ps_c_pool,
                tc.tile_pool(name="ps_r", bufs=1, space="PSUM") as ps_r_pool,
                tc.tile_pool(name="ps3", bufs=2, space="PSUM") as ps3,
            ):
                wout_sb = wo.tile([P, n_dt, O_OUT], mm_dt)
                nc.sync.dma_start(wout_sb[:], wout_r)
                sub_w = 256  # stage-3 token sub-chunk (DMA/SBUF granularity)
                LAG = 2  # chunks of attention emitted before a chunk's
                # projection, so the PE stream never blocks on a gather

                def proj_chunk(ch):
                    b, qs = divmod(ch, n_qs)
                    q_lo = b * seq + qs * qs_w
                    for sub in range(qs_w // sub_w):
                        t_lo = q_lo + sub * sub_w
                        ctx_t = ctxs.tile(
                            [P, n_dt, sub_w], mm_dt, name="ctx_t"
                        )
                        nc.sync.dma_start(
                            ctx_t[:],
                            cc_out_r[ch][:, :, sub * sub_w : (sub + 1) * sub_w],
                        )
                        for ot in range(O_OUT // P):
                            ps_o = ps3.tile([P, sub_w], FP32, name="ps_o")
                            for dt in range(n_dt):
                                MM(
                                    ps_o[:],
                                    wout_sb[:, dt, ot * P : (ot + 1) * P],
                                    ctx_t[:, dt, :],
                                    start=(dt == 0),
                                    stop=(dt == n_dt - 1),
                                )
                            out_t = outs.tile([P, sub_w], FP32, name="out_t")
                            nc.scalar.activation(
                                out_t[:],
                                ps_o[:],
                                mybir.ActivationFunctionType.Identity,
                                bias=b_out_sb[:, ot : ot + 1],
                            )
                            nc.sync.dma_start(
                                out_r[:, ot, t_lo : t_lo + sub_w], out_t[:]
                            )

                for ch in range(n_ch):
                    b, qs = divmod(ch, n_qs)
                    q_lo = b * seq + qs * qs_w
                    ctx_ch = ctxp.tile([P, HPC, qs_w], mm_dt, name="ctx_ch")
                    for h in range(HPC):
                        ps_ctx = ps_c_pool.tile([P, qs_w], FP32)
                        ps_sum = ps_r_pool.tile([1, qs_w], FP32)
                        for kt in range(n_kt):
                            k_lo = b * seq + kt * P
                            ps_sc = ps_s_pool.tile([P, qs_w], FP32)
                            MM(
                                ps_sc[:],
                                qk_sb[:, HPC + h, k_lo : k_lo + P],
                                qk_sb[:, h, q_lo : q_lo + qs_w],
                                start=True,
                                stop=True,
                            )
                            exp_t = exps.tile([P, qs_w], mm_dt)
                            nc.scalar.activation(
                                exp_t[:],
                                ps_sc[:],
                                mybir.ActivationFunctionType.Exp,
                                scale=SCALE,
                            )
                            MM(
                                ps_ctx[:],
                                v_sb[:, (b * seq) // P + kt, h * HD : (h + 1) * HD],
                                exp_t[:],
                                start=(kt == 0),
                                stop=(kt == n_kt - 1),
                            )
                            # denominator: accumulate column sums on the PE
                            MM(
                                ps_sum[:],
                                ones_col[:],
                                exp_t[:],
                                start=(kt == 0),
                                stop=(kt == n_kt - 1),
                            )
                        inv = sums.tile([1, qs_w], FP32)
                        nc.vector.reciprocal(inv[:], ps_sum[:])
                        ps_b = ps_r_pool.tile([P, qs_w], FP32)
                        nc.tensor.matmul(
                            ps_b[:], ones_row[:], inv[:], start=True, stop=True
                        )
                        invb = sums.tile([P, qs_w], FP32)
                        nc.vector.tensor_copy(invb[:], ps_b[:])
                        nc.vector.tensor_mul(
                            ctx_ch[:, h, :], ps_ctx[:], invb[:]
                        )
                    # ship this chunk's context and gather across cores
                    nc.sync.dma_start(cc_in_r[ch], ctx_ch[:])
                    nc.gpsimd.collective_compute(
                        "AllGather",
                        mybir.AluOpType.bypass,
                        replica_groups=[list(range(N_CORES))],
                        ins=[cc_in.ap()[ch]],
                        outs=[cc_out.ap()[ch]],
                    )
                    # lagged output projection keeps PE off the gather path
                    if ch >= LAG:
                        proj_chunk(ch - LAG)
                for ch in range(n_ch - LAG, n_ch):
                    proj_chunk(ch)

            acts_scope.__exit__(None, None, None)

    return nc


def _make_in_maps(hidden_states, w_qkv, b_qkv, w_out, b_out):
    b, s, _ = hidden_states.shape
    t_all = b * s
    x = np.ascontiguousarray(
        hidden_states.reshape(t_all, H).T, dtype=np.float32
    )  # [H, T]
    in_maps = []
    for c in range(N_CORES):
        h0 = HPC * c
        q_rows = np.r_[h0 * HD : (h0 + HPC) * HD]
        k_rows = H + q_rows
        v_rows = 2 * H + q_rows
        qk_rows = np.r_[q_rows, k_rows]
        w1t_qk = np.ascontiguousarray(w_qkv[qk_rows, :].T, dtype=np.float32)
        w1t_v = np.ascontiguousarray(w_qkv[v_rows, :].T, dtype=np.float32)
        b_qk = np.ascontiguousarray(
            b_qkv[qk_rows].reshape(O_QK // P, P).T, dtype=np.float32
        )
        b_v = np.ascontiguousarray(
            np.broadcast_to(b_qkv[v_rows], (P, O_V)), dtype=np.float32
        )
        o_lo = c * O_OUT
        wout_t = np.ascontiguousarray(
            w_out[o_lo : o_lo + O_OUT, :].T, dtype=np.float32
        )
        b_o = np.ascontiguousarray(
            b_out[o_lo : o_lo + O_OUT].reshape(O_OUT // P, P).T,
            dtype=np.float32,
        )
        in_maps.append(
            {
                "ones_d": np.ones((P, 1), dtype=np.float32),
                "xt": x,
                "w1t_qk": w1t_qk,
                "w1t_v": w1t_v,
                "b_qk": b_qk,
                "b_v": b_v,
                "wout_t": wout_t,
                "b_out": b_o,
            }
        )
    return in_maps


_program_cache = {}


def _get_program(seq=S, mm_dt=MM_DT):
    key = (seq, mm_dt)
    if key not in _program_cache:
        _program_cache[key] = _build_program(seq, mm_dt)
    return _program_cache[key]


def run(hidden_states, w_qkv, b_qkv, w_out, b_out, trace=False, mm_dt=MM_DT):
    """Run the sharded kernel; returns (output, BassKernelResults)."""
    b, s, _ = hidden_states.shape
    nc = _get_program(s, mm_dt)
    in_maps = _make_in_maps(hidden_states, w_qkv, b_qkv, w_out, b_out)
    res = run_bass_kernel_spmd(
        nc, in_maps, list(range(N_CORES)), trace=trace
    )
    # per-core output is out^T [O_OUT, T]; stack to [H, T] then transpose
    cols = np.concatenate([res.results[c]["out"] for c in range(N_CORES)], axis=0)
    return (
        np.ascontiguousarray(cols.T).reshape(b, s, H).astype(np.float32),
        res,
    )


def kernel(hidden_states, w_qkv, b_qkv, w_out, b_out):
    out, _ = run(
        np.asarray(hidden_states),
        np.asarray(w_qkv),
        np.asarray(b_qkv),
        np.asarray(w_out),
        np.asarray(b_out),
    )
    return out
